# revision 1
# baseline (speedup 1.0000x reference)
"""Cross-head online Hadamard (32-point WHT across attention heads).

Input x: (4, 4096, 4096) fp32. hidden 4096 = 32 heads x 128 head_dim.
For every (token, head_dim) pair, apply a 32-point Walsh-Hadamard
transform across the 32 heads, scaled by 1/sqrt(32).

Strategy (pure data parallel over tokens, 8 cores):
  - Each core gets 2048 tokens (rows of the flattened (16384, 4096) view).
  - Per 128-token tile, a gather-DMA lays SBUF partitions out as
    p = g*32 + h (4 token-groups x 32 heads); the free axis is
    (token-within-group, head_dim) -> contiguous 512B runs in DRAM.
  - One 128x128 block-diagonal matrix (4 copies of the 32x32 Hadamard,
    1/sqrt(32) folded in) multiplies the tile on the TensorEngine in
    N=512 chunks (fp32, exact). DVE copies PSUM->SBUF, scatter-DMA
    writes back in the same layout.
"""

import numpy as np

HEAD_DIM = 128
N_HEADS = 32
HIDDEN = N_HEADS * HEAD_DIM  # 4096
N_CORES = 8
T_TOTAL = 4 * 4096  # 16384 tokens
T_CORE = T_TOTAL // N_CORES  # 2048
GROUPS = 4  # token groups stacked on the 128 partitions
TILE_TOK = 128  # tokens per SBUF tile
J = TILE_TOK // GROUPS  # tokens per group within a tile
FREE = J * HEAD_DIM  # fp32 elements per partition per tile
MM_N = 512  # matmul moving-dim chunk (one PSUM bank, fp32 max)
BUFS_IN = 4
BUFS_OUT = 4
COPY_SPLIT = 0  # every COPY_SPLIT-th PSUM copy goes to scalar engine (0=off)

_NC_CACHE = {}


def _hadamard_butterfly_matrix() -> np.ndarray:
    """The exact matrix of reference._matmul_hadU on a length-32 vector,
    extracted by pushing the identity through the same butterfly."""
    n = N_HEADS
    y = np.eye(n, dtype=np.float64)[:, :, None]  # (B=n, n, 1)
    while y.shape[1] > 1:
        m, c = y.shape[1] // 2, y.shape[2]
        y = y.reshape(n, m, 2, c)
        a, b = y[:, :, 0, :], y[:, :, 1, :]
        y = np.stack([a + b, a - b], axis=2).reshape(n, m, 2 * c)
    out = y.reshape(n, n)  # row i = f(e_i) -> M = out.T
    return out.T


def _weights() -> np.ndarray:
    """128x128 block-diagonal lhsT for out = lhsT.T @ rhs (4 head-groups)."""
    m = _hadamard_butterfly_matrix() * np.float64(np.float32(1.0 / np.sqrt(np.float32(N_HEADS))))
    lhst_block = m.T  # lhsT[k, m] = M[m, k]; symmetric for Sylvester order
    w = np.zeros((128, 128), dtype=np.float64)
    for g in range(GROUPS):
        w[g * N_HEADS:(g + 1) * N_HEADS, g * N_HEADS:(g + 1) * N_HEADS] = lhst_block
    return w.astype(np.float32)


def _build_nc(passes: int = 1):
    """passes>1 repeats the whole transform into a scratch DRAM tensor
    (bench-only, amortizes dispatch overhead); the last pass writes y."""
    import concourse.mybir as mybir
    import concourse.tile as tile
    from concourse import bacc

    nc = bacc.Bacc("TRN2", target_bir_lowering=False, debug=False,
                   num_devices=N_CORES)
    x = nc.dram_tensor("x", [T_CORE, HIDDEN], mybir.dt.float32,
                       kind="ExternalInput").ap()
    w = nc.dram_tensor("w", [128, 128], mybir.dt.float32,
                       kind="ExternalInput").ap()
    y = nc.dram_tensor("y", [T_CORE, HIDDEN], mybir.dt.float32,
                       kind="ExternalOutput").ap()
    scr = None
    if passes > 1:
        scr = nc.dram_tensor("scr", [T_CORE, HIDDEN], mybir.dt.float32).ap()

    ntiles = T_CORE // TILE_TOK
    f32 = mybir.dt.float32

    with tile.TileContext(nc) as tc:
        with tc.tile_pool(name="wpool", bufs=1) as wp, \
             tc.tile_pool(name="tin", bufs=BUFS_IN) as pin, \
             tc.tile_pool(name="tout", bufs=BUFS_OUT) as pout, \
             tc.tile_pool(name="ps", bufs=8, space="PSUM") as pps:
            w_t = wp.tile([128, 128], f32)
            nc.sync.dma_start(out=w_t[:], in_=w)
            for p in range(passes):
                out_dram = y if p == passes - 1 else scr
                for i in range(ntiles):
                    base = i * TILE_TOK
                    t_in = pin.tile([128, FREE], f32, tag="tin")
                    # one DMA per token-group: 3-dim AP (h, j, d), 512B runs
                    for g in range(GROUPS):
                        xin = x[base + g * J:base + (g + 1) * J].rearrange(
                            "j (h d) -> h j d", h=N_HEADS)
                        nc.sync.dma_start(
                            out=t_in[g * N_HEADS:(g + 1) * N_HEADS, :], in_=xin)
                    t_out = pout.tile([128, FREE], f32, tag="tout")
                    for m in range(FREE // MM_N):
                        ps = pps.tile([128, MM_N], f32, tag="ps")
                        nc.tensor.matmul(ps[:], w_t[:],
                                         t_in[:, m * MM_N:(m + 1) * MM_N],
                                         start=True, stop=True)
                        cp = (nc.scalar.copy if COPY_SPLIT and m % COPY_SPLIT == 0
                              else nc.vector.tensor_copy)
                        cp(out=t_out[:, m * MM_N:(m + 1) * MM_N], in_=ps[:])
                    for g in range(GROUPS):
                        yout = out_dram[base + g * J:base + (g + 1) * J].rearrange(
                            "j (h d) -> h j d", h=N_HEADS)
                        nc.scalar.dma_start(
                            out=yout, in_=t_out[g * N_HEADS:(g + 1) * N_HEADS, :])
    nc.compile()
    return nc


def _get_nc(passes: int = 1):
    key = ("nc", passes)
    if key not in _NC_CACHE:
        _NC_CACHE[key] = _build_nc(passes)
    return _NC_CACHE[key]


def run(inputs: dict, trace: bool = False, trace_cores=None):
    """Run on 8 NeuronCores; returns (full_output, BassKernelResults)."""
    import os

    from concourse.bass_utils import run_bass_kernel_spmd

    if not trace:
        # NTFF tracing needs antenv.axon_hooks, absent in this axon client;
        # a stray BASS_TRACE=1 in the environment would crash the run.
        os.environ["BASS_NEVER_TRACE"] = "1"

    x = np.ascontiguousarray(np.asarray(inputs["x"], dtype=np.float32))
    init_shape = x.shape
    xf = x.reshape(-1, HIDDEN)
    assert xf.shape[0] == T_TOTAL, f"expected {T_TOTAL} tokens, got {xf.shape[0]}"
    w = _weights()
    in_maps = [{"x": np.ascontiguousarray(xf[c * T_CORE:(c + 1) * T_CORE]),
                "w": w} for c in range(N_CORES)]
    res = run_bass_kernel_spmd(
        _get_nc(), in_maps, core_ids=list(range(N_CORES)),
        trace=trace, trace_cores=trace_cores)
    y = np.concatenate([r["y"] for r in res.results], axis=0)
    return y.reshape(init_shape), res


def kernel(**inputs) -> np.ndarray:
    out, _ = run(inputs)
    return out



# revision 2
# speedup vs baseline: 2.4763x; 2.4763x over previous
"""Cross-head online Hadamard (32-point WHT across attention heads).

Input x: (4, 4096, 4096) fp32. hidden 4096 = 32 heads x 128 head_dim.
For every (token, head_dim) pair, apply a 32-point Walsh-Hadamard
transform across the 32 heads, scaled by 1/sqrt(32).

Strategy (pure data parallel over tokens, 8 cores):
  - Each core gets 2048 tokens (rows of the flattened (16384, 4096) view).
  - fp16 on the wire: the host casts x to fp16 before upload and casts the
    fp16 result back to fp32 after download. This halves HBM traffic
    (the kernel is DMA-bound); the 2e-2 rel-err budget dwarfs the ~4e-4
    fp16 rounding error. The matmul accumulates in fp32 PSUM.
  - Per 128-token tile, gather-DMAs lay SBUF partitions out as
    p = g*32 + h (4 token-groups x 32 heads); the free axis is
    (token-within-group, head_dim) -> 256B runs in DRAM. Loads and
    stores alternate between the two HWDGE rings (sync / scalar) --
    descriptor generation is the bottleneck for this run size, and
    splitting across both rings nearly doubles throughput.
  - One 128x128 block-diagonal fp16 matrix (4 copies of the 32x32
    Hadamard, 1/sqrt(32) folded in) multiplies the tile on the
    TensorEngine in N=512 chunks. PSUM(fp32)->SBUF(fp16) copies
    alternate between the vector and scalar engines.
"""

import numpy as np

HEAD_DIM = 128
N_HEADS = 32
HIDDEN = N_HEADS * HEAD_DIM  # 4096
N_CORES = 8
T_TOTAL = 4 * 4096  # 16384 tokens
T_CORE = T_TOTAL // N_CORES  # 2048
GROUPS = 4  # token groups stacked on the 128 partitions
TILE_TOK = 128  # tokens per SBUF tile
J = TILE_TOK // GROUPS  # tokens per group within a tile
FREE = J * HEAD_DIM  # elements per partition per tile
MM_N = 512  # matmul moving-dim chunk (one PSUM bank, fp32)
BUFS_IN = 4
BUFS_OUT = 4

_NC_CACHE = {}


def _hadamard_butterfly_matrix() -> np.ndarray:
    """The exact matrix of reference._matmul_hadU on a length-32 vector,
    extracted by pushing the identity through the same butterfly."""
    n = N_HEADS
    y = np.eye(n, dtype=np.float64)[:, :, None]  # (B=n, n, 1)
    while y.shape[1] > 1:
        m, c = y.shape[1] // 2, y.shape[2]
        y = y.reshape(n, m, 2, c)
        a, b = y[:, :, 0, :], y[:, :, 1, :]
        y = np.stack([a + b, a - b], axis=2).reshape(n, m, 2 * c)
    out = y.reshape(n, n)  # row i = f(e_i) -> M = out.T
    return out.T


def _weights() -> np.ndarray:
    """128x128 block-diagonal fp16 lhsT for out = lhsT.T @ rhs."""
    m = _hadamard_butterfly_matrix() * np.float64(
        np.float32(1.0 / np.sqrt(np.float32(N_HEADS))))
    lhst_block = m.T  # lhsT[k, m] = M[m, k]; symmetric for Sylvester order
    w = np.zeros((128, 128), dtype=np.float64)
    for g in range(GROUPS):
        w[g * N_HEADS:(g + 1) * N_HEADS,
          g * N_HEADS:(g + 1) * N_HEADS] = lhst_block
    return w.astype(np.float16)


def _build_nc(passes: int = 1):
    """passes>1 repeats the whole transform into a scratch DRAM tensor
    (bench-only, amortizes dispatch overhead); the last pass writes y."""
    import concourse.mybir as mybir
    import concourse.tile as tile
    from concourse import bacc

    nc = bacc.Bacc("TRN2", target_bir_lowering=False, debug=False,
                   num_devices=N_CORES)
    f16 = mybir.dt.float16
    f32 = mybir.dt.float32
    x = nc.dram_tensor("x", [T_CORE, HIDDEN], f16, kind="ExternalInput").ap()
    w = nc.dram_tensor("w", [128, 128], f16, kind="ExternalInput").ap()
    y = nc.dram_tensor("y", [T_CORE, HIDDEN], f16, kind="ExternalOutput").ap()
    scr = None
    if passes > 1:
        scr = nc.dram_tensor("scr", [T_CORE, HIDDEN], f16).ap()

    ntiles = T_CORE // TILE_TOK

    with tile.TileContext(nc) as tc:
        with tc.tile_pool(name="wpool", bufs=1) as wp, \
             tc.tile_pool(name="tin", bufs=BUFS_IN) as pin, \
             tc.tile_pool(name="tout", bufs=BUFS_OUT) as pout, \
             tc.tile_pool(name="ps", bufs=8, space="PSUM") as pps:
            w_t = wp.tile([128, 128], f16)
            nc.sync.dma_start(out=w_t[:], in_=w)
            for p in range(passes):
                out_dram = y if p == passes - 1 else scr
                for i in range(ntiles):
                    base = i * TILE_TOK
                    t_in = pin.tile([128, FREE], f16, tag="tin")
                    # one DMA per token-group: 3-dim AP (h, j, d), 256B runs;
                    # alternate between the two HWDGE rings
                    for g in range(GROUPS):
                        eng = nc.sync if g % 2 == 0 else nc.scalar
                        xin = x[base + g * J:base + (g + 1) * J].rearrange(
                            "j (h d) -> h j d", h=N_HEADS)
                        eng.dma_start(
                            out=t_in[g * N_HEADS:(g + 1) * N_HEADS, :], in_=xin)
                    t_out = pout.tile([128, FREE], f16, tag="tout")
                    for m in range(FREE // MM_N):
                        ps = pps.tile([128, MM_N], f32, tag="ps")
                        nc.tensor.matmul(ps[:], w_t[:],
                                         t_in[:, m * MM_N:(m + 1) * MM_N],
                                         start=True, stop=True)
                        cp = (nc.vector.tensor_copy if m % 2 == 0
                              else nc.scalar.copy)
                        cp(out=t_out[:, m * MM_N:(m + 1) * MM_N], in_=ps[:])
                    for g in range(GROUPS):
                        eng = nc.scalar if g % 2 == 0 else nc.sync
                        yout = out_dram[base + g * J:base + (g + 1) * J].rearrange(
                            "j (h d) -> h j d", h=N_HEADS)
                        eng.dma_start(
                            out=yout, in_=t_out[g * N_HEADS:(g + 1) * N_HEADS, :])
    nc.compile()
    return nc


def _get_nc(passes: int = 1):
    key = ("nc", passes)
    if key not in _NC_CACHE:
        _NC_CACHE[key] = _build_nc(passes)
    return _NC_CACHE[key]


def run(inputs: dict, trace: bool = False, trace_cores=None):
    """Run on 8 NeuronCores; returns (full_output, BassKernelResults)."""
    import os

    from concourse.bass_utils import run_bass_kernel_spmd

    if not trace:
        # NTFF tracing needs antenv.axon_hooks, absent in this axon client;
        # a stray BASS_TRACE=1 in the environment would crash the run.
        os.environ["BASS_NEVER_TRACE"] = "1"

    x = np.asarray(inputs["x"])
    init_shape = x.shape
    xf = np.ascontiguousarray(x.reshape(-1, HIDDEN).astype(np.float16))
    assert xf.shape[0] == T_TOTAL, f"expected {T_TOTAL} tokens, got {xf.shape[0]}"
    w = _weights()
    in_maps = [{"x": xf[c * T_CORE:(c + 1) * T_CORE],
                "w": w} for c in range(N_CORES)]
    res = run_bass_kernel_spmd(
        _get_nc(), in_maps, core_ids=list(range(N_CORES)),
        trace=trace, trace_cores=trace_cores)
    y = np.concatenate([r["y"] for r in res.results], axis=0)
    return y.astype(np.float32).reshape(init_shape), res


def kernel(**inputs) -> np.ndarray:
    out, _ = run(inputs)
    return out
